# revision 14
# baseline (speedup 1.0000x reference)
"""Multi-head attention (B=2, S=2048, H=1024, 16 heads x 64d) on 8 trn2 cores.

Sharding: tensor-parallel over heads (2 heads/core). Each core computes the
qkv projection for its 384 output features, attention for its 2 heads, and a
partial o_proj ([4096,1024] over its 128-feature slice) in f16. Host sums the
8 partials and adds b_o.

v2 layout/schedule (per core, feature-major):
  QT/KT [128, 4096] f16, rows = head_local*64 + d, cols = b*2048 + s.
  V is produced TOKEN-MAJOR directly (Vaug[tok, b, h, kslab, 65]): per
  128-token block, 8 accumulating matmuls with the x chunk as the stationary
  operand plus a K=1 ones-row matmul that adds the bias; no PE transposes.
  Head-PAIRED attention units (b, qc): per k-slab the two heads' score
  matmuls are K=64 row-tiles (tile_position (0,0)/(64,0)) that run
  concurrently; one exp covers both heads' [128,1024] scores from PSUM; PV
  accumulates [65, 2, 512] (ones-column in Vaug gives softmax denominators).
  Normalizer: o_ps evacuated to SBUF fast (frees the single PSUM buffer),
  reciprocal of the sums row via ACT Log + Exp(scale=-1) (same table set as
  the softmax exp; DVE's iterative reciprocal is ~8 cyc/elem), ones-matmul
  broadcast, DVE mul into OT. pv/norm software-pipeline across pair
  boundaries (carry list); a fine-grained filler queue (qkv halves / V
  blocks / o_proj token tiles / norm tails) drains inside exp shadows.
"""
import sys

sys.path.insert(0, "/opt/trn_rl_repo")
import numpy as np

NHEADS = 16
HEAD_DIM = 64
HIDDEN = 1024
QKV = NHEADS * HEAD_DIM  # 1024
SCALING = HEAD_DIM ** -0.5
B = 2
S = 2048
T = B * S  # 4096
NCORES = 8
HPC = NHEADS // NCORES  # 2 heads per core
FEAT = HPC * HEAD_DIM  # 128
CHUNK = 512
NCHUNK = S // CHUNK  # 4 per batch
KSLABS = HIDDEN // 128  # 8
SSLABS = S // 128  # 16
D1 = HEAD_DIM + 1  # 65
LAG = 4  # pv_group emission lag (groups) behind s_group

_CACHE = {}
LAST_RESULT = None  # BassKernelResults of the most recent kernel() call


def _split_waits(nc, keep=1):
    """Hoist excess per-instruction sem waits into standalone EventSemaphore
    instructions (walrus codegen has small per-opcode wait budgets)."""
    import bass_rust
    import concourse.mybir as mybir

    n_new = 0
    for f in nc.m.functions:
        for blk in f.blocks:
            out = []
            changed = False
            for inst in blk.instructions:
                si = inst.sync_info
                waits = list(si.on_wait) if si is not None else []
                if len(waits) > keep:
                    excess = waits[: len(waits) - keep]
                    kept = waits[len(waits) - keep:]
                    for w in excess:
                        out.append(mybir.InstEventSemaphore(
                            name=f"{inst.name}-esw{n_new}",
                            engine=inst.engine,
                            sync_info=bass_rust.SyncInfo(on_wait=[w], on_update=[]),
                        ))
                        n_new += 1
                    inst.sync_info = bass_rust.SyncInfo(
                        on_wait=kept, on_update=list(si.on_update))
                    changed = True
                out.append(inst)
            if changed:
                blk.instructions = out
    return n_new


def _build(reps=1):
    import concourse.bass as bass
    import concourse.mybir as mybir
    import concourse.tile as tile

    f32 = mybir.dt.float32
    f32r = mybir.dt.float32r
    f16 = mybir.dt.float16
    Exp = mybir.ActivationFunctionType.Exp
    Log = mybir.ActivationFunctionType.Ln

    nc = bass.Bass()
    xT = nc.dram_tensor("xT", [HIDDEN, T], f16, kind="ExternalInput")
    wqkvT = nc.dram_tensor("wqkvT", [HIDDEN, 3 * FEAT], f16, kind="ExternalInput")
    bqkv = nc.dram_tensor("bqkv", [FEAT, 3], f32, kind="ExternalInput")
    bvT = nc.dram_tensor("bvT", [1, FEAT], f16, kind="ExternalInput")
    woT = nc.dram_tensor("woT", [FEAT, HIDDEN], f16, kind="ExternalInput")
    out_d = nc.dram_tensor("out", [T, HIDDEN], f16, kind="ExternalOutput")

    with tile.TileContext(nc) as tc, nc.allow_low_precision(reason="f16 matmuls"):
        with (
            tc.tile_pool(name="sing", bufs=1) as sing,
            tc.tile_pool(name="xp", bufs=2) as xp,
            tc.tile_pool(name="pp", bufs=8) as pp,
            tc.tile_pool(name="stg", bufs=2) as stg,
            tc.tile_pool(name="sm", bufs=2) as sm,
            tc.tile_pool(name="op", bufs=2) as op,
            tc.tile_pool(name="ps_s", bufs=2, space="PSUM") as ps_s,
            tc.tile_pool(name="ps_o", bufs=1, space="PSUM") as ps_o,
            tc.tile_pool(name="ps_mm", bufs=2, space="PSUM") as ps_mm,
        ):
            wq_sb = sing.tile([128, KSLABS, 3 * FEAT], f16, tag="wq")
            wo_sb = sing.tile([FEAT, HIDDEN], f16, tag="wo")
            bq_sb = sing.tile([FEAT, 3], f32, tag="bq")
            bv_sb = sing.tile([1, FEAT], f16, tag="bv")
            ones1 = sing.tile([1, HEAD_DIM], f32r, tag="on")
            QT = sing.tile([128, T], f16, tag="qt")
            KT = sing.tile([128, T], f16, tag="kt")
            OT = sing.tile([128, T], f16, tag="ot")
            Vaug = sing.tile([128, B, HPC, SSLABS, D1], f16, tag="va")

            nc.sync.dma_start(
                out=wq_sb, in_=wqkvT[:].rearrange("(s p) f -> p s f", p=128))
            nc.sync.dma_start(out=wo_sb, in_=woT[:])
            nc.sync.dma_start(out=bq_sb, in_=bqkv[:])
            nc.sync.dma_start(out=bv_sb, in_=bvT[:])
            ones_f = stg.tile([1, HEAD_DIM], f32, tag="onf")
            nc.vector.memset(ones_f, 1.0)
            nc.vector.tensor_copy(ones1, ones_f)
            onerow = sing.tile([1, 128], f16, tag="or")
            nc.vector.memset(onerow, 1.0)
            vst = stg.tile([128, B * HPC * SSLABS], f16, tag="vst")
            nc.vector.memset(vst, 1.0)
            nc.vector.tensor_copy(Vaug[:, :, :, :, HEAD_DIM:D1], vst)
            # warm the exp/log table set while qkv runs (one-time ~2.7us)
            scr = stg.tile([1, 64], f16, tag="scr")
            nc.scalar.activation(out=scr, in_=ones_f, func=Exp)
            nc.scalar.activation(out=scr, in_=ones_f, func=Log)

            xT_c = xT[:].rearrange("(s p) t -> p s t", p=128)

            from collections import deque
            filler = deque()
            done_tokens = set()
            bstate = {"allowed": 0.0, "spent": 0.0}
            GROUP_BUDGET = 525.0  # ns of filler PE time per attention group

            def mark(tok):
                done_tokens.add(tok)

            def pop_one():
                # zero-cost thunks (DMA/alloc/ACT-only) ride along for free
                while filler and filler[0][0] == 0:
                    filler.popleft()[1]()
                if filler:
                    cost, fn = filler.popleft()
                    bstate["spent"] += cost
                    fn()

            def drain_budget():
                # pop filler until this group's cumulative PE budget is used
                bstate["allowed"] += GROUP_BUDGET
                while filler and bstate["spent"] < bstate["allowed"]:
                    pop_one()
                while filler and filler[0][0] == 0:
                    filler.popleft()[1]()

            def drain_all():
                while filler:
                    pop_one()

            def need(tok):
                # force-drain until the producer thunk for `tok` has been
                # emitted (emission order must respect data order for Tile's
                # dependency tracking)
                while tok not in done_tokens:
                    assert filler, f"no producer queued for {tok}"
                    pop_one()

            def qkv_dma(b, n):
                g = b * NCHUNK + n
                xc = xp.tile([128, KSLABS, CHUNK], f16, tag="xc", name="xc")
                nc.sync.dma_start(out=xc, in_=xT_c[:, :, g * CHUNK:(g + 1) * CHUNK])
                return xc

            def qk_feat_quarter(b, n, m, q, xc, acc_box):
                g = b * NCHUNK + n
                if q == 0:
                    acc_box["acc"] = ps_mm.tile(
                        [128, CHUNK], f32, tag="mm", name="acc")
                acc = acc_box["acc"]
                for s in range(2 * q, 2 * q + 2):
                    nc.tensor.matmul(
                        acc, wq_sb[:, s, m * FEAT:(m + 1) * FEAT], xc[:, s, :],
                        start=(s == 0), stop=(s == KSLABS - 1))
                if q == 3:
                    dest = (QT, KT)[m]
                    lo, hi = g * CHUNK, (g + 1) * CHUNK
                    nc.vector.tensor_scalar_add(
                        dest[:, lo:hi], acc, bq_sb[:, m:m + 1])
                    mark(("QK"[m], b, n))

            def v_block_half(b, n, j, half, xc, acc_box):
                # token-major V for token block 128*(4n+j): both heads at once
                k = 4 * n + j
                if half == 0:
                    acc_box["acc"] = ps_mm.tile(
                        [128, CHUNK], f32, tag="mm", name="accv")
                acc = acc_box["acc"]
                for s in range(4 * half, 4 * half + 4):
                    nc.tensor.matmul(
                        acc[:, 0:128], xc[:, s, 128 * j:128 * (j + 1)],
                        wq_sb[:, s, 2 * FEAT:3 * FEAT],
                        start=(s == 0), stop=False)
                if half == 1:
                    nc.tensor.matmul(
                        acc[:, 0:128], onerow, bv_sb,
                        start=False, stop=True)
                    nc.vector.tensor_copy(
                        Vaug[:, b, 0:HPC, k, 0:HEAD_DIM],
                        acc[:, 0:128].rearrange("p (h d) -> p h d", h=HPC))
                    mark(("V", b, k))

            def qkv_chunk_dense(b, n):
                xc = qkv_dma(b, n)
                for m in range(2):
                    box = {}
                    for q in range(4):
                        qk_feat_quarter(b, n, m, q, xc, box)
                for j in range(4):
                    box = {}
                    v_block_half(b, n, j, 0, xc, box)
                    v_block_half(b, n, j, 1, xc, box)

            def qkv_chunk_thunks(b, n):
                xc_box = {}
                ths = [(0, lambda: xc_box.__setitem__("xc", qkv_dma(b, n)))]
                for m in (1, 0):  # K first: consumed earlier than Q
                    acc_box = {}
                    for q in range(4):
                        ths.append((430, lambda m=m, q=q, ab=acc_box:
                                    qk_feat_quarter(b, n, m, q, xc_box["xc"], ab)))
                for j in range(4):
                    acc_box = {}
                    for half in range(2):
                        ths.append((360, lambda j=j, half=half, ab=acc_box:
                                    v_block_half(b, n, j, half, xc_box["xc"], ab)))
                return ths

            def s_group(b, qc, k):
                need(("Q", b, qc))
                need(("K", b, k // 4))
                qlo = b * S + qc * CHUNK
                s_ps = ps_s.tile([128, HPC, CHUNK], f32, tag="s", name="s_ps")
                for h in range(HPC):
                    nc.tensor.matmul(
                        s_ps[:, h, :],
                        KT[64 * h:64 * h + 64,
                           b * S + 128 * k: b * S + 128 * (k + 1)],
                        QT[64 * h:64 * h + 64, qlo:qlo + CHUNK],
                        start=True, stop=True)
                pt = pp.tile([128, HPC, CHUNK], f16, tag="pt", name="pt")
                nc.scalar.activation(out=pt, in_=s_ps, func=Exp)
                return pt

            def pv_group(o_ps, b, qc, k, pt):
                need(("V", b, k))
                for h in range(HPC):
                    nc.tensor.matmul(
                        o_ps[:, h, :], Vaug[:, b, h, k, :], pt[:, h, :],
                        start=(k == 0), stop=(k == SSLABS - 1))

            def norm_carry(o_ps, box):
                def t_copy():
                    # evacuate PSUM fast so the single o_ps buffer frees
                    box["o_sb"] = sm.tile([D1, HPC, CHUNK], f32, tag="osb",
                                          name="o_sb")
                    nc.vector.tensor_copy(box["o_sb"], o_ps)

                return [t_copy]

            def norm_rec(box):
                # 1/sums = exp(-log(sums)); Ln+Exp share one table set
                lns = sm.tile([1, HPC, CHUNK], f32, tag="lns", name="lns")
                nc.scalar.activation(
                    out=lns, in_=box["o_sb"][HEAD_DIM:D1, :, :], func=Log)
                box["rec"] = sm.tile([1, HPC, CHUNK], f32r, tag="rec",
                                     name="rec")
                nc.scalar.activation(
                    out=box["rec"], in_=lns, func=Exp, scale=-1.0)

            def norm_head(b, qc, box, h):
                qlo = b * S + qc * CHUNK
                b_ps = ps_mm.tile([HEAD_DIM, CHUNK], f32, tag="mm", name="bps")
                nc.tensor.matmul(
                    b_ps, ones1, box["rec"][:, h, :], start=True, stop=True)
                nc.vector.tensor_mul(
                    OT[64 * h:64 * h + 64, qlo:qlo + CHUNK],
                    box["o_sb"][0:HEAD_DIM, h, :], b_ps)

            carry = []
            pair_seq = [0]

            def attn_pair(b, qc, after_carry=None):
                nonlocal carry
                # the carry's t_copy reuses the 2-deep o_sb pool: the pair-2
                # norm readers must be emitted before that buffer rotates
                if pair_seq[0] >= 2:
                    need(("N", pair_seq[0] - 2))
                for c in carry:
                    c()
                carry = []
                if after_carry is not None:
                    after_carry()
                # allocate AFTER the carry flush: the previous pair's norm
                # copy (emitted just above) must order before this reuse
                o_ps = ps_o.tile([D1, HPC, CHUNK], f32, tag="o", name="o_ps")
                pts = {}
                for kk in range(0, SSLABS, 2):
                    pts[kk] = s_group(b, qc, kk)
                    pts[kk + 1] = s_group(b, qc, kk + 1)
                    drain_budget()
                    drain_budget()
                    for k in (kk, kk + 1):
                        if k >= LAG:
                            pv_group(o_ps, b, qc, k - LAG, pts.pop(k - LAG))
                nbox = {}
                carry = [
                    (lambda k=k: pv_group(o_ps, b, qc, k, pts.pop(k)))
                    for k in range(SSLABS - LAG, SSLABS)
                ] + norm_carry(o_ps, nbox)
                # norm ACT ops + PE tails ride the filler queue: the ACT pair
                # lands between softmax exps, the tiny bMM/mul never stall the
                # in-order PE stream at a pair boundary
                p = pair_seq[0]
                filler.append((0, lambda: norm_rec(nbox)))
                filler.append((220, lambda: norm_head(b, qc, nbox, 0)))
                filler.append((220, lambda: (norm_head(b, qc, nbox, 1),
                                             mark(("N", p)))))
                pair_seq[0] += 1

            def oproj_pair(j, jj, ost):
                t = 4 * j + jj
                for nh in range(HIDDEN // CHUNK):
                    acc = ps_mm.tile([128, CHUNK], f32, tag="mm", name="acc2")
                    nc.tensor.matmul(
                        acc, OT[:, 128 * t:128 * (t + 1)],
                        wo_sb[:, nh * CHUNK:(nh + 1) * CHUNK],
                        start=True, stop=True)
                    nc.vector.tensor_copy(
                        ost[:, jj, nh * CHUNK:(nh + 1) * CHUNK], acc)

            def oproj_group_thunks(j):
                box = {}

                def alloc():
                    box["ost"] = op.tile([128, 4, HIDDEN], f16, tag="ost",
                                         name="ost")

                thunks = [(0, alloc)]
                for jj in range(4):
                    thunks.append(
                        (430, lambda jj=jj: oproj_pair(j, jj, box["ost"])))

                def flush():
                    nc.sync.dma_start(
                        out=out_d[512 * j:512 * (j + 1), :].rearrange(
                            "(jj p) h -> p jj h", p=128),
                        in_=box["ost"])

                thunks.append((0, flush))
                return thunks

            # ---- emission ----
            # reps share SBUF state and identical input values, so qkv of
            # rep r+1 overlaps rep r's attention tail via the filler queue
            # (done_tokens persist: rewrites land behind the prior readers)
            pair_list = [(b, qc) for b in range(B) for qc in range(NCHUNK)]
            prev_group = [None]

            def after_carry(pg=prev_group):
                if pg[0] is not None:
                    filler.extend(oproj_group_thunks(pg[0]))

            for _rep in range(reps):
                if _rep == 0:
                    qkv_chunk_dense(0, 0)
                    chunks = [(0, 1), (0, 2), (0, 3)]
                else:
                    chunks = [(0, n) for n in range(NCHUNK)]
                for (cb, cn) in chunks + [(1, n) for n in range(NCHUNK)]:
                    filler.extend(qkv_chunk_thunks(cb, cn))

                for (b, qc) in pair_list:
                    attn_pair(b, qc, after_carry=after_carry)
                    prev_group[0] = b * NCHUNK + qc
            for c in carry:
                c()
            carry = []
            filler.extend(oproj_group_thunks(prev_group[0]))
            drain_all()

    _split_waits(nc)
    return nc


def make_in_maps(inputs):
    return _make_in_maps(
        inputs["hidden_states"], inputs["w_qkv"], inputs["b_qkv"],
        inputs["w_o"], inputs["b_o"])


def _make_in_maps(hidden_states, w_qkv, b_qkv, w_o, b_o):
    x16 = np.ascontiguousarray(
        np.asarray(hidden_states, dtype=np.float32).reshape(T, HIDDEN).T
    ).astype(np.float16)
    w_qkv = np.asarray(w_qkv, dtype=np.float32)
    b_qkv = np.asarray(b_qkv, dtype=np.float32)
    w_o = np.asarray(w_o, dtype=np.float32)

    in_maps = []
    for c in range(NCORES):
        rq = slice(c * FEAT, (c + 1) * FEAT)
        wq = w_qkv[0:QKV][rq] * SCALING
        wk = w_qkv[QKV:2 * QKV][rq]
        wv = w_qkv[2 * QKV:3 * QKV][rq]
        bq = b_qkv[0:QKV][rq] * SCALING
        bk = b_qkv[QKV:2 * QKV][rq]
        bv = b_qkv[2 * QKV:3 * QKV][rq]
        in_maps.append({
            "xT": x16,
            "wqkvT": np.ascontiguousarray(
                np.concatenate([wq, wk, wv], axis=0).T).astype(np.float16),
            "bqkv": np.ascontiguousarray(np.stack([bq, bk, bv], axis=1)),
            "bvT": np.ascontiguousarray(bv.reshape(1, FEAT)).astype(np.float16),
            "woT": np.ascontiguousarray(w_o[:, rq].T).astype(np.float16),
        })
    return in_maps


def kernel(hidden_states, w_qkv, b_qkv, w_o, b_o):
    global LAST_RESULT
    from concourse.bass_utils import run_bass_kernel_spmd
    import os

    if "nc" not in _CACHE:
        _CACHE["nc"] = _build()
    nc = _CACHE["nc"]

    in_maps = _make_in_maps(hidden_states, w_qkv, b_qkv, w_o, b_o)
    b_o = np.asarray(b_o, dtype=np.float32)

    trace = bool(os.environ.get("KERNEL_TRACE"))
    res = run_bass_kernel_spmd(nc, in_maps, list(range(NCORES)), trace=trace)
    LAST_RESULT = res

    acc = np.zeros((T, HIDDEN), dtype=np.float64)
    for c in range(NCORES):
        acc += res.results[c]["out"]
    out = (acc + b_o).astype(np.float32).reshape(B, S, HIDDEN)
    return out
